# revision 1
# baseline (speedup 1.0000x reference)
"""Causal multi-head self-attention on 8 Trainium2 NeuronCores.

Problem: x[4,2048,1024], 16 heads of dim 64, causal softmax attention,
output projection Wo[1024,1024].

Sharding: core c handles batch b=c//2 and head-group g=c%2 (8 heads).
Each core computes attention for its 8 heads plus the partial output
projection over its 512 columns of the concat dim; the host sums the two
partials per batch. x is transposed on the host (input marshaling) so the
contraction dim lands on SBUF partitions without on-chip transposes.

Single software-pipelined loop over the four 512-wide q-chunks (all
matmuls fp32r = fp32 bits at ~bf16 PE speed):
  per chunk c: DMA xT s-chunk; V[s,8*64]+ones column ("V|1"); QT/KT
  [2heads*64, 512] for 4 head pairs; then attention: scoresT[k,q] =
  KT.T @ QT (two heads row-tiled in the PE array), exp on ScalarE
  (softmax without max subtraction: scores bounded ~8), triangular mask
  only on diagonal tiles, attnT[65,q] += [V|1].T @ probsT (row 64 =
  softmax denominator), normalize by broadcast(1/denom) into the dead
  qt storage of this chunk. The previous chunk's output projection
  y[q,e] = sum_p attnT_p.T @ WoT_p is interleaved into the exp-bound
  attention pipeline in small bursts; the next chunk's V/QT/KT fill the
  remaining PE gaps (scheduler runs them when the PE would otherwise
  idle on ScalarE).
"""

import sys

if "/opt/trn_rl_repo" not in sys.path:
    sys.path.insert(0, "/opt/trn_rl_repo")

import os

import numpy as np

import concourse.mybir as mybir
import concourse.tile as tile
from concourse import bacc

F32 = mybir.dt.float32
F32R = mybir.dt.float32r
EXP = mybir.ActivationFunctionType.Exp

B, S, D, H, DK = 4, 2048, 1024, 16, 64
NP = 4        # head pairs per core (8 heads)
DC = 8        # 128-row chunks of D
ST = 16       # 128-row tiles of S
SC = 4        # 512-col chunks of S
QW = 512      # q-chunk width

_cache = {}


def _build(repeat=1):
    scp_bufs = int(os.environ.get("K_SCP_BUFS", "2"))
    att_bufs = int(os.environ.get("K_ATT_BUFS", "2"))
    gap_bufs = int(os.environ.get("K_GAP_BUFS", "2"))
    pr_bufs = int(os.environ.get("K_PR_BUFS", "3"))
    wo_every = int(os.environ.get("K_WO_EVERY", "4"))

    nc = bacc.Bacc("TRN2", debug=False)
    xtd = nc.dram_tensor("xtd", [D, S], F32, kind="ExternalInput").ap()
    wq = nc.dram_tensor("wq", [D, 512], F32, kind="ExternalInput").ap()
    wk = nc.dram_tensor("wk", [D, 512], F32, kind="ExternalInput").ap()
    wv = nc.dram_tensor("wv", [D, 512], F32, kind="ExternalInput").ap()
    wot = nc.dram_tensor("wot", [512, D], F32, kind="ExternalInput").ap()
    y = nc.dram_tensor("y", [S, D], F32, kind="ExternalOutput").ap()

    with tile.TileContext(nc) as tc:
        with (
            tc.tile_pool(name="const", bufs=1) as cpool,
            tc.tile_pool(name="persist", bufs=1) as pers,
            tc.tile_pool(name="w", bufs=1) as wpool,
            tc.tile_pool(name="xt", bufs=1) as xt_pool,
            tc.tile_pool(name="probs", bufs=pr_bufs) as pr_pool,
            tc.tile_pool(name="small", bufs=2) as sm_pool,
            tc.tile_pool(name="yout", bufs=2) as y_pool,
            tc.tile_pool(name="ps", bufs=1, space="PSUM") as psall,
        ):
            # upper-triangular (f >= p) keep-mask for diagonal score tiles
            trimask = cpool.tile([128, 128], F32, tag="trimask")
            nc.gpsimd.memset(trimask[:], 1.0)
            nc.gpsimd.affine_select(
                out=trimask[:],
                in_=trimask[:],
                compare_op=mybir.AluOpType.is_ge,
                fill=0.0,
                base=0,
                pattern=[[1, 128]],
                channel_multiplier=-1,
            )
            ones8 = cpool.tile([128, 8, 1], F32, tag="ones8")
            nc.gpsimd.memset(ones8[:], 1.0)

            wq_sb = wpool.tile([128, DC, 512], F32R, tag="wq")
            wk_sb = wpool.tile([128, DC, 512], F32R, tag="wk")
            wv_sb = wpool.tile([128, DC, 512], F32R, tag="wv")
            wot_sb = wpool.tile([128, NP, D], F32R, tag="wot")
            wqr = wq.rearrange("(a p) n -> p a n", p=128).bitcast(F32R)
            wkr = wk.rearrange("(a p) n -> p a n", p=128).bitcast(F32R)
            wvr = wv.rearrange("(a p) n -> p a n", p=128).bitcast(F32R)
            for d in range(DC):
                nc.sync.dma_start(wv_sb[:, d, :], wvr[:, d, :])
            _first_xts = {}

            def _load_xts(c, pool=None):
                ts = [
                    pool.tile([128, QW], F32R, tag=f"xt{d}", name=f"xt{d}")
                    for d in range(DC)
                ]
                for d in range(DC):
                    nc.sync.dma_start(
                        ts[d][:],
                        xtd[
                            128 * d : 128 * (d + 1), QW * c : QW * (c + 1)
                        ].bitcast(F32R),
                    )
                return ts

            _xts_boot = _load_xts(0, xt_pool)
            for d in range(DC):
                nc.sync.dma_start(wq_sb[:, d, :], wqr[:, d, :])
                nc.sync.dma_start(wk_sb[:, d, :], wkr[:, d, :])
            wotr = wot.rearrange("(a p) n -> p a n", p=128).bitcast(F32R)
            for pp in range(NP):
                nc.sync.dma_start(wot_sb[:, pp, :], wotr[:, pp, :])

            for _rep in range(repeat):
                qt = [
                    pers.tile([128, S], F32R, tag=f"qt{p}", name=f"qt{p}")
                    for p in range(NP)
                ]
                kt = [
                    pers.tile([128, S], F32R, tag=f"kt{p}", name=f"kt{p}")
                    for p in range(NP)
                ]
                vaug = [
                    pers.tile([128, 8, 65], F32R, tag=f"va{st}", name=f"va{st}")
                    for st in range(ST)
                ]
                # normalized attention output reuses the dead q-chunk storage
                attnT = qt

                def emit_wo(c):
                    """Output projection of q-chunk c, one (t, eh) burst of 4
                    PE matmuls per advance -- small enough that the 2-deep
                    scores buffer keeps ScalarE fed through each burst."""
                    for t4 in range(4):
                        t = 4 * c + t4
                        ysb = y_pool.tile([128, D], F32, tag="ysb")
                        for eh in (0, 1):
                            yps = psall.tile(
                                [128, 512], F32, tag="gap", bufs=gap_bufs
                            )
                            for p in range(NP):
                                nc.tensor.matmul(
                                    yps[:],
                                    attnT[p][:, 128 * t : 128 * (t + 1)],
                                    wot_sb[:, p, 512 * eh : 512 * (eh + 1)],
                                    start=(p == 0),
                                    stop=(p == NP - 1),
                                )
                            nc.vector.tensor_copy(
                                ysb[:, 512 * eh : 512 * (eh + 1)], yps[:]
                            )
                            yield
                        nc.sync.dma_start(y[128 * t : 128 * (t + 1), :], ysb[:])
                    while True:
                        yield

                xts_cur = _xts_boot if _rep == 0 else None
                _xts_boot = None
                for c in range(SC):
                    # ---- produce this chunk's xT columns, V, QT, KT ----
                    xts = xts_cur if xts_cur is not None else _load_xts(0, xt_pool)
                    for st4 in range(4):
                        st = 4 * c + st4
                        vps = psall.tile([128, 512], F32, tag="gap", bufs=gap_bufs)
                        for d in range(DC):
                            nc.tensor.matmul(
                                vps[:],
                                xts[d][:, 128 * st4 : 128 * (st4 + 1)],
                                wv_sb[:, d, :],
                                start=(d == 0),
                                stop=(d == DC - 1),
                            )
                        va = vaug[st]
                        nc.vector.tensor_copy(
                            va[:, :, 0:64],
                            vps[:].rearrange("p (h k) -> p h k", h=8),
                        )
                        nc.vector.tensor_copy(va[:, :, 64:65], ones8[:])
                    for p in range(NP):
                        qps = psall.tile([128, 512], F32, tag="gap", bufs=gap_bufs)
                        for d in range(DC):
                            nc.tensor.matmul(
                                qps[:],
                                wq_sb[:, d, 128 * p : 128 * (p + 1)],
                                xts[d][:],
                                start=(d == 0),
                                stop=(d == DC - 1),
                            )
                        nc.vector.tensor_copy(qt[p][:, QW * c : QW * (c + 1)], qps[:])
                        kps = psall.tile([128, 512], F32, tag="gap", bufs=gap_bufs)
                        for d in range(DC):
                            nc.tensor.matmul(
                                kps[:],
                                wk_sb[:, d, 128 * p : 128 * (p + 1)],
                                xts[d][:],
                                start=(d == 0),
                                stop=(d == DC - 1),
                            )
                        nc.vector.tensor_copy(kt[p][:, QW * c : QW * (c + 1)], kps[:])
                    xts_cur = _load_xts(c + 1, xt_pool) if c + 1 < SC else None

                    # ---- attention for q-chunk c (+ interleaved Wo of c-1) ----
                    wo_gen = emit_wo(c - 1) if c > 0 else None
                    it = 0
                    nkt = 4 * c + 4
                    for p in range(NP):
                        a0 = psall.tile([65, 512], F32, tag="att", bufs=att_bufs)
                        a1 = psall.tile([65, 512], F32, tag="att", bufs=att_bufs)
                        for k in range(nkt):
                            j = k - 4 * c
                            # cols q < 128*j of this q-chunk are strictly future
                            # for this k-tile: skip them everywhere. (k==0 covers
                            # the full range, so every PSUM element of the
                            # accumulation is initialized.)
                            lo = 128 * j if j > 0 else 0
                            scp = psall.tile(
                                [128, 1024], F32, tag="scp", bufs=scp_bufs
                            )
                            for hh in (0, 1):
                                nc.tensor.matmul(
                                    scp[:, 512 * hh + lo : 512 * (hh + 1)],
                                    kt[p][
                                        64 * hh : 64 * (hh + 1),
                                        128 * k : 128 * (k + 1),
                                    ],
                                    qt[p][
                                        64 * hh : 64 * (hh + 1),
                                        QW * c + lo : QW * (c + 1),
                                    ],
                                    start=True,
                                    stop=True,
                                    tile_position=(64 * hh, 0),
                                )
                            pr = pr_pool.tile([128, 1024], F32R, tag="pr")
                            if lo:
                                # boundary: exp only the live q-range of both
                                # head-halves in one strided 3D op
                                nc.scalar.activation(
                                    pr[:].rearrange("p (h q) -> p h q", h=2)[
                                        :, :, lo:512
                                    ],
                                    scp[:].rearrange("p (h q) -> p h q", h=2)[
                                        :, :, lo:512
                                    ],
                                    EXP,
                                    scale=0.125,
                                )
                            else:
                                nc.scalar.activation(pr[:], scp[:], EXP, scale=0.125)
                            if j >= 0:
                                # diagonal block: triangular keep-mask
                                for hh in (0, 1):
                                    off = 512 * hh + 128 * j
                                    nc.vector.tensor_mul(
                                        pr[:, off : off + 128],
                                        pr[:, off : off + 128],
                                        trimask[:],
                                    )
                            for hh, aps in ((0, a0), (1, a1)):
                                nc.tensor.matmul(
                                    aps[:, lo:512],
                                    vaug[k][:, 2 * p + hh, :],
                                    pr[:, 512 * hh + lo : 512 * (hh + 1)],
                                    start=(k == 0),
                                    stop=(k == nkt - 1),
                                )
                            it += 1
                            if wo_gen is not None and it % wo_every == 0:
                                next(wo_gen)
                        for hh, aps in ((0, a0), (1, a1)):
                            rc = sm_pool.tile([1, 512], F32, tag="rc")
                            nc.vector.reciprocal(rc[:], aps[64:65, :])
                            rb = sm_pool.tile([64, 512], F32, tag="rb")
                            nc.gpsimd.partition_broadcast(rb[:], rc[:])
                            nc.vector.tensor_mul(
                                attnT[p][
                                    64 * hh : 64 * (hh + 1), QW * c : QW * (c + 1)
                                ],
                                aps[0:64, :],
                                rb[:],
                            )
                    if wo_gen is not None:
                        for _ in range(10):
                            next(wo_gen)
                # final chunk's projection
                wo_gen = emit_wo(SC - 1)
                for _ in range(10):
                    next(wo_gen)

    nc.compile()
    return nc


def _in_maps(x, Wq, Wk, Wv, Wo):
    xts = [np.ascontiguousarray(x[b].T, dtype=np.float32) for b in range(B)]
    maps = []
    for c in range(8):
        b, g = c // 2, c % 2
        hs = slice(8 * g, 8 * (g + 1))
        maps.append(
            {
                "xtd": xts[b],
                "wq": np.ascontiguousarray(
                    Wq[hs].transpose(1, 0, 2).reshape(D, 512), dtype=np.float32
                ),
                "wk": np.ascontiguousarray(
                    Wk[hs].transpose(1, 0, 2).reshape(D, 512), dtype=np.float32
                ),
                "wv": np.ascontiguousarray(
                    Wv[hs].transpose(1, 0, 2).reshape(D, 512), dtype=np.float32
                ),
                "wot": np.ascontiguousarray(
                    Wo[:, 512 * g : 512 * (g + 1)].T, dtype=np.float32
                ),
            }
        )
    return maps


def _make_runner(repeat=1):
    """Compile the Bass program and build a cached 8-core jitted callable."""
    import jax
    from jax.experimental.shard_map import shard_map
    from jax.sharding import Mesh, NamedSharding, PartitionSpec

    import concourse.mybir as _mybir
    from concourse import bass2jax

    nc = _build(repeat=repeat)
    bass2jax.install_neuronx_cc_hook()

    partition_name = nc.partition_id_tensor.name if nc.partition_id_tensor else None
    in_names, out_names, out_avals = [], [], []
    for alloc in nc.m.functions[0].allocations:
        if not isinstance(alloc, _mybir.MemoryLocationSet):
            continue
        name = alloc.memorylocations[0].name
        if alloc.kind == "ExternalInput":
            if name != partition_name:
                in_names.append(name)
        elif alloc.kind == "ExternalOutput":
            out_names.append(name)
            out_avals.append(
                jax.core.ShapedArray(
                    tuple(alloc.tensor_shape), _mybir.dt.np(alloc.dtype)
                )
            )
    n_params = len(in_names)
    all_in_names = list(in_names) + list(out_names)
    if partition_name is not None:
        all_in_names.append(partition_name)

    def _body(*args):
        operands = list(args)
        if partition_name is not None:
            operands.append(bass2jax.partition_id_tensor())
        outs = bass2jax._bass_exec_p.bind(
            *operands,
            out_avals=tuple(out_avals),
            in_names=tuple(all_in_names),
            out_names=tuple(out_names),
            lowering_input_output_aliases=(),
            sim_require_finite=True,
            sim_require_nnan=True,
            nc=nc,
        )
        return tuple(outs)

    n_outs = len(out_names)
    donate = tuple(range(n_params, n_params + n_outs))
    devices = jax.devices()[:8]
    mesh = Mesh(np.asarray(devices), ("core",))
    spec = NamedSharding(mesh, PartitionSpec("core"))
    sharded = jax.jit(
        shard_map(
            _body,
            mesh=mesh,
            in_specs=(PartitionSpec("core"),) * (n_params + n_outs),
            out_specs=(PartitionSpec("core"),) * n_outs,
            check_rep=False,
        ),
        donate_argnums=donate,
        keep_unused=True,
    )
    return {
        "nc": nc,
        "sharded": sharded,
        "in_names": in_names,
        "out_names": out_names,
        "out_avals": out_avals,
        "spec": spec,
    }


def kernel(x, Wq, Wk, Wv, Wo, _time_runs=0):
    import time

    import jax

    x, Wq, Wk, Wv, Wo = (np.asarray(a, dtype=np.float32) for a in (x, Wq, Wk, Wv, Wo))
    if "runner" not in _cache:
        _cache["runner"] = _make_runner()
    r = _cache["runner"]
    maps = _in_maps(x, Wq, Wk, Wv, Wo)
    concat_in = [
        np.concatenate([maps[c][name] for c in range(8)], axis=0)
        for name in r["in_names"]
    ]
    dev_in = [jax.device_put(a, r["spec"]) for a in concat_in]

    def zeros():
        return [
            jax.device_put(
                np.zeros((8 * av.shape[0], *av.shape[1:]), av.dtype), r["spec"]
            )
            for av in r["out_avals"]
        ]

    out = r["sharded"](*dev_in, *zeros())
    jax.block_until_ready(out)
    if _time_runs:
        times = []
        for _ in range(_time_runs):
            z = zeros()
            jax.block_until_ready(z)
            t0 = time.perf_counter()
            out = r["sharded"](*dev_in, *z)
            jax.block_until_ready(out)
            times.append(time.perf_counter() - t0)
        _cache["exec_times_s"] = times
    yi = r["out_names"].index("y")
    y_all = np.asarray(out[yi]).reshape(8, S, D)
    yf = np.empty((B, S, D), dtype=np.float32)
    for b in range(B):
        yf[b] = y_all[2 * b] + y_all[2 * b + 1]
    return yf



# revision 12
# speedup vs baseline: 1.0509x; 1.0509x over previous
"""Causal multi-head self-attention on 8 Trainium2 NeuronCores.

Problem: x[4,2048,1024], 16 heads of dim 64, causal softmax attention,
output projection Wo[1024,1024].

Sharding: core c handles batch b=c//2 and head-group g=c%2 (8 heads).
Each core computes attention for its 8 heads plus the partial output
projection over its 512 columns of the concat dim; the host sums the two
partials per batch. x is transposed on the host (input marshaling) so the
contraction dim lands on SBUF partitions without on-chip transposes.

Per-chunk pipeline over four 512-wide q-chunks. QKV projections are fp32r
(full-width matmuls, 1 cyc/row); QT/KT/probs/V are stored bf16. Scores
per (pair, k-tile): two matmuls [128s, 512q] with the two heads packed
into PE row-halves (tile_position), exp on ScalarE straight into bf16
probsT. The attention-apply is transposed vs the usual layout: out
[128 q, 65] += probsT_tile.T @ [V|1], costing 65 PE rows per matmul
instead of 512 (the cost model charges output free size only), with the
softmax denominator riding in column 64. Normalization is one
stride-0-broadcast reciprocal multiply into a bf16 staging tile, which a
PE transpose flips back to [dk, q] for the (unchanged, fp32r) output
projection. A unified filler queue interleaves Wo(c-1) bursts, QKV(c+1)
chains and the deferred normalize/transposes into the ScalarE-bound
attention loop so the PE never idles on exp.
"""

import sys

if "/opt/trn_rl_repo" not in sys.path:
    sys.path.insert(0, "/opt/trn_rl_repo")

import os
from collections import deque

import numpy as np

import concourse.mybir as mybir
import concourse.tile as tile
from concourse import bacc
from concourse.bass import broadcast_tensor_aps
from concourse.masks import make_identity

F32 = mybir.dt.float32
F32R = mybir.dt.float32r
BF16 = mybir.dt.bfloat16
EXP = mybir.ActivationFunctionType.Exp

B, S, D, H, DK = 4, 2048, 1024, 16, 64
NP = 4        # head pairs per core (8 heads)
DC = 8        # 128-row chunks of D
ST = 16       # 128-row tiles of S
SC = 4        # 512-col chunks of S
QW = 512      # q-chunk width

_cache = {}


def _build(repeat=1):
    scp_bufs = int(os.environ.get("K_SCP_BUFS", "2"))
    pr_bufs = int(os.environ.get("K_PR_BUFS", "3"))
    gap_bufs = int(os.environ.get("K_GAP_BUFS", "2"))
    xt_bufs = int(os.environ.get("K_XT_BUFS", "2"))
    fil_every = int(os.environ.get("K_FIL_EVERY", "1"))

    nc = bacc.Bacc("TRN2", debug=False)
    xtd = nc.dram_tensor("xtd", [D, S], F32, kind="ExternalInput").ap()
    wq = nc.dram_tensor("wq", [D, 512], F32, kind="ExternalInput").ap()
    wk = nc.dram_tensor("wk", [D, 512], F32, kind="ExternalInput").ap()
    wv = nc.dram_tensor("wv", [D, 512], F32, kind="ExternalInput").ap()
    wot = nc.dram_tensor("wot", [512, D], BF16, kind="ExternalInput").ap()
    y = nc.dram_tensor("y", [S, D], F32, kind="ExternalOutput").ap()

    with tile.TileContext(nc) as tc:
        with (
            tc.tile_pool(name="const", bufs=1) as cpool,
            tc.tile_pool(name="persist", bufs=1) as pers,
            tc.tile_pool(name="w", bufs=1) as wpool,
            tc.tile_pool(name="xt", bufs=xt_bufs) as xt_pool,
            tc.tile_pool(name="probs", bufs=pr_bufs) as pr_pool,
            tc.tile_pool(name="small", bufs=2) as sm_pool,
            tc.tile_pool(name="abf", bufs=2) as ab_pool,
            tc.tile_pool(name="yout", bufs=2) as y_pool,
            tc.tile_pool(name="ps", bufs=1, space="PSUM") as psall,
        ):
            # upper-triangular (f >= p) keep-mask for diagonal score tiles
            trimask = cpool.tile([128, 128], BF16, tag="trimask")
            nc.gpsimd.memset(trimask[:], 1.0)
            nc.gpsimd.affine_select(
                out=trimask[:],
                in_=trimask[:],
                compare_op=mybir.AluOpType.is_ge,
                fill=0.0,
                base=0,
                pattern=[[1, 128]],
                channel_multiplier=-1,
            )
            # identity for PE transposes
            ident = cpool.tile([128, 128], BF16, tag="ident")
            make_identity(nc, ident[:])

            wq_sb = wpool.tile([128, DC, 512], F32R, tag="wq")
            wk_sb = wpool.tile([128, DC, 512], F32R, tag="wk")
            wv_sb = wpool.tile([128, DC, 512], F32R, tag="wv")
            wot_sb = wpool.tile([128, NP, D], BF16, tag="wot")
            wqr = wq.rearrange("(a p) n -> p a n", p=128).bitcast(F32R)
            wkr = wk.rearrange("(a p) n -> p a n", p=128).bitcast(F32R)
            wvr = wv.rearrange("(a p) n -> p a n", p=128).bitcast(F32R)
            for d in range(DC):
                nc.sync.dma_start(wv_sb[:, d, :], wvr[:, d, :])

            def _load_xts(c):
                ts = [
                    xt_pool.tile([128, QW], F32R, tag=f"xt{d}", name=f"xt{d}")
                    for d in range(DC)
                ]
                for d in range(DC):
                    nc.sync.dma_start(
                        ts[d][:],
                        xtd[
                            128 * d : 128 * (d + 1), QW * c : QW * (c + 1)
                        ].bitcast(F32R),
                    )
                return ts

            xts_by_chunk = {0: _load_xts(0)}
            for d in range(DC):
                nc.sync.dma_start(wq_sb[:, d, :], wqr[:, d, :])
                nc.sync.dma_start(wk_sb[:, d, :], wkr[:, d, :])
            wotr = wot.rearrange("(a p) n -> p a n", p=128)
            for pp in range(NP):
                nc.sync.dma_start(wot_sb[:, pp, :], wotr[:, pp, :])

            for _rep in range(repeat):
                qt = [
                    pers.tile([128, S], BF16, tag=f"qt{p}", name=f"qt{p}")
                    for p in range(NP)
                ]
                kt = [
                    pers.tile([128, S], BF16, tag=f"kt{p}", name=f"kt{p}")
                    for p in range(NP)
                ]
                attnT = [
                    pers.tile([128, S], BF16, tag=f"at{p}", name=f"at{p}")
                    for p in range(NP)
                ]
                vaug = [
                    pers.tile([128, 8, 65], BF16, tag=f"va{st}", name=f"va{st}")
                    for st in range(ST)
                ]
                for st in range(ST):
                    nc.gpsimd.memset(vaug[st][:, :, 64:65], 1.0)

                def emit_wo(c):
                    """Output projection of q-chunk c as 8 filler units of 4
                    fp32r matmuls each."""
                    for t4 in range(4):
                        t = 4 * c + t4
                        ysb = y_pool.tile([128, D], F32, tag="ysb")
                        for eh in (0, 1):
                            yps = psall.tile(
                                [128, 512], F32, tag="gap", bufs=gap_bufs
                            )
                            for p in range(NP):
                                nc.tensor.matmul(
                                    yps[:],
                                    attnT[p][:, 128 * t : 128 * (t + 1)],
                                    wot_sb[:, p, 512 * eh : 512 * (eh + 1)],
                                    start=(p == 0),
                                    stop=(p == NP - 1),
                                )
                            nc.vector.tensor_copy(
                                ysb[:, 512 * eh : 512 * (eh + 1)], yps[:]
                            )
                            yield
                        nc.sync.dma_start(y[128 * t : 128 * (t + 1), :], ysb[:])

                def emit_qkv(c, xts):
                    """V/Q/K projections for chunk c as 12 filler units of 8
                    chained fp32r matmuls each."""
                    for st4 in range(4):
                        st = 4 * c + st4
                        vps = psall.tile(
                            [128, 512], F32, tag="gap", bufs=gap_bufs
                        )
                        for d in range(DC):
                            nc.tensor.matmul(
                                vps[:],
                                xts[d][:, 128 * st4 : 128 * (st4 + 1)],
                                wv_sb[:, d, :],
                                start=(d == 0),
                                stop=(d == DC - 1),
                            )
                        nc.vector.tensor_copy(
                            vaug[st][:, :, 0:64],
                            vps[:].rearrange("p (h k) -> p h k", h=8),
                        )
                        yield
                    for p in range(NP):
                        qps = psall.tile(
                            [128, 512], F32, tag="gap", bufs=gap_bufs
                        )
                        for d in range(DC):
                            nc.tensor.matmul(
                                qps[:],
                                wq_sb[:, d, 128 * p : 128 * (p + 1)],
                                xts[d][:],
                                start=(d == 0),
                                stop=(d == DC - 1),
                            )
                        nc.vector.tensor_copy(qt[p][:, QW * c : QW * (c + 1)], qps[:])
                        yield
                        kps = psall.tile(
                            [128, 512], F32, tag="gap", bufs=gap_bufs
                        )
                        for d in range(DC):
                            nc.tensor.matmul(
                                kps[:],
                                wk_sb[:, d, 128 * p : 128 * (p + 1)],
                                xts[d][:],
                                start=(d == 0),
                                stop=(d == DC - 1),
                            )
                        nc.vector.tensor_copy(kt[p][:, QW * c : QW * (c + 1)], kps[:])
                        yield

                # chunk 0's QKV runs straight (nothing to hide it behind)
                for _ in emit_qkv(0, xts_by_chunk[0]):
                    pass
                if SC > 1:
                    xts_by_chunk[1] = _load_xts(1)

                units = deque()

                def pump_one():
                    # advance the head generator by one unit (round-robin)
                    while units:
                        g = units.popleft()
                        try:
                            next(g[1])
                            units.append(g)
                            return
                        except StopIteration:
                            continue

                def emit_transposes(p, c, abf):
                    for t4 in range(4):
                        tp = psall.tile([128, 512], F32, tag="gap", bufs=gap_bufs)
                        tpb = tp[:, 0:64].bitcast(BF16)
                        nc.tensor.transpose(tpb, abf[:, :, t4, :], ident[:])
                        nc.vector.tensor_copy(
                            attnT[p][:, QW * c + 128 * t4 : QW * c + 128 * (t4 + 1)],
                            tpb,
                        )
                        yield

                for c in range(SC):
                    if c + 2 < SC:
                        xts_by_chunk[c + 2] = _load_xts(c + 2)
                    if c > 0:
                        units.append(("wo", emit_wo(c - 1)))
                    if c + 1 < SC:
                        units.append(("qkv", emit_qkv(c + 1, xts_by_chunk[c + 1])))

                    nkt = 4 * c + 4
                    it = 0
                    for p in range(NP):
                        # [q, (hh, t4, col)] accumulators; col 64 = denominator
                        att = psall.tile(
                            [128, 2, 4, 128], F32, tag="att", bufs=1, name="att"
                        )
                        for k in range(nkt):
                            j = k - 4 * c
                            lo = 128 * j if j > 0 else 0
                            scp = psall.tile(
                                [128, 1024], F32, tag="scp", bufs=scp_bufs
                            )
                            for hh in (0, 1):
                                nc.tensor.matmul(
                                    scp[:, 512 * hh + lo : 512 * (hh + 1)],
                                    kt[p][
                                        64 * hh : 64 * (hh + 1),
                                        128 * k : 128 * (k + 1),
                                    ],
                                    qt[p][
                                        64 * hh : 64 * (hh + 1),
                                        QW * c + lo : QW * (c + 1),
                                    ],
                                    start=True,
                                    stop=True,
                                    tile_position=(64 * hh, 0),
                                )
                            pr = pr_pool.tile([128, 1024], BF16, tag="pr")
                            if lo:
                                nc.scalar.activation(
                                    pr[:].rearrange("p (h q) -> p h q", h=2)[
                                        :, :, lo:512
                                    ],
                                    scp[:].rearrange("p (h q) -> p h q", h=2)[
                                        :, :, lo:512
                                    ],
                                    EXP,
                                    scale=0.125,
                                )
                            else:
                                nc.scalar.activation(pr[:], scp[:], EXP, scale=0.125)
                            if j >= 0:
                                for hh in (0, 1):
                                    off = 512 * hh + 128 * j
                                    nc.vector.tensor_mul(
                                        pr[:, off : off + 128],
                                        pr[:, off : off + 128],
                                        trimask[:],
                                    )
                            # one accumulation group per PSUM bank (= per hh):
                            # start only on the bank's first write, stop only
                            # on its last; the bank's lazy zeroing makes the
                            # later t4 streams read-as-zero on first touch.
                            for t4 in range(max(j, 0), 4):
                                for hh in (0, 1):
                                    nc.tensor.matmul(
                                        att[:, hh, t4, 0:65],
                                        pr[
                                            :,
                                            512 * hh + 128 * t4 : 512 * hh
                                            + 128 * (t4 + 1),
                                        ],
                                        vaug[k][:, 2 * p + hh, :],
                                        start=(k == 0 and t4 == 0),
                                        stop=(j == 3 and t4 == 3),
                                    )
                            it += 1
                            if it % fil_every == 0:
                                pump_one()
                        # normalize pair p: one reciprocal + one broadcast mul
                        rc = sm_pool.tile([128, 2, 4, 1], F32, tag="rc")
                        nc.vector.reciprocal(rc[:], att[:, :, :, 64:65])
                        abf = ab_pool.tile([128, 2, 4, 64], BF16, tag="abf")
                        in0 = att[:, :, :, 0:64]
                        in1, _ = broadcast_tensor_aps(rc[:], in0)
                        nc.vector.tensor_mul(abf[:], in0, in1)
                        # transposes run as filler during the next pair
                        units.appendleft(("tr", emit_transposes(p, c, abf)))
                        pump_one()
                    # drain all filler before the next chunk's scores
                    for _ in range(40):
                        pump_one()
                # final chunk's projection
                for _ in emit_wo(SC - 1):
                    pass

    nc.compile()
    return nc


def _in_maps(x, Wq, Wk, Wv, Wo):
    xts = [np.ascontiguousarray(x[b].T, dtype=np.float32) for b in range(B)]
    maps = []
    for c in range(8):
        b, g = c // 2, c % 2
        hs = slice(8 * g, 8 * (g + 1))
        maps.append(
            {
                "xtd": xts[b],
                "wq": np.ascontiguousarray(
                    Wq[hs].transpose(1, 0, 2).reshape(D, 512), dtype=np.float32
                ),
                "wk": np.ascontiguousarray(
                    Wk[hs].transpose(1, 0, 2).reshape(D, 512), dtype=np.float32
                ),
                "wv": np.ascontiguousarray(
                    Wv[hs].transpose(1, 0, 2).reshape(D, 512), dtype=np.float32
                ),
                "wot": np.ascontiguousarray(
                    Wo[:, 512 * g : 512 * (g + 1)].T
                ).astype(mybir.dt.np(mybir.dt.bfloat16)),
            }
        )
    return maps


def _make_runner(repeat=1):
    """Compile the Bass program and build a cached 8-core jitted callable."""
    import jax
    from jax.experimental.shard_map import shard_map
    from jax.sharding import Mesh, NamedSharding, PartitionSpec

    import concourse.mybir as _mybir
    from concourse import bass2jax

    nc = _build(repeat=repeat)
    bass2jax.install_neuronx_cc_hook()

    partition_name = nc.partition_id_tensor.name if nc.partition_id_tensor else None
    in_names, out_names, out_avals = [], [], []
    for alloc in nc.m.functions[0].allocations:
        if not isinstance(alloc, _mybir.MemoryLocationSet):
            continue
        name = alloc.memorylocations[0].name
        if alloc.kind == "ExternalInput":
            if name != partition_name:
                in_names.append(name)
        elif alloc.kind == "ExternalOutput":
            out_names.append(name)
            out_avals.append(
                jax.core.ShapedArray(
                    tuple(alloc.tensor_shape), _mybir.dt.np(alloc.dtype)
                )
            )
    n_params = len(in_names)
    all_in_names = list(in_names) + list(out_names)
    if partition_name is not None:
        all_in_names.append(partition_name)

    def _body(*args):
        operands = list(args)
        if partition_name is not None:
            operands.append(bass2jax.partition_id_tensor())
        outs = bass2jax._bass_exec_p.bind(
            *operands,
            out_avals=tuple(out_avals),
            in_names=tuple(all_in_names),
            out_names=tuple(out_names),
            lowering_input_output_aliases=(),
            sim_require_finite=True,
            sim_require_nnan=True,
            nc=nc,
        )
        return tuple(outs)

    n_outs = len(out_names)
    donate = tuple(range(n_params, n_params + n_outs))
    devices = jax.devices()[:8]
    mesh = Mesh(np.asarray(devices), ("core",))
    spec = NamedSharding(mesh, PartitionSpec("core"))
    sharded = jax.jit(
        shard_map(
            _body,
            mesh=mesh,
            in_specs=(PartitionSpec("core"),) * (n_params + n_outs),
            out_specs=(PartitionSpec("core"),) * n_outs,
            check_rep=False,
        ),
        donate_argnums=donate,
        keep_unused=True,
    )
    return {
        "nc": nc,
        "sharded": sharded,
        "in_names": in_names,
        "out_names": out_names,
        "out_avals": out_avals,
        "spec": spec,
    }


def kernel(x, Wq, Wk, Wv, Wo, _time_runs=0):
    import time

    import jax

    x, Wq, Wk, Wv, Wo = (np.asarray(a, dtype=np.float32) for a in (x, Wq, Wk, Wv, Wo))
    if "runner" not in _cache:
        _cache["runner"] = _make_runner()
    r = _cache["runner"]
    maps = _in_maps(x, Wq, Wk, Wv, Wo)
    concat_in = [
        np.concatenate([maps[c][name] for c in range(8)], axis=0)
        for name in r["in_names"]
    ]
    dev_in = [jax.device_put(a, r["spec"]) for a in concat_in]

    def zeros():
        return [
            jax.device_put(
                np.zeros((8 * av.shape[0], *av.shape[1:]), av.dtype), r["spec"]
            )
            for av in r["out_avals"]
        ]

    out = r["sharded"](*dev_in, *zeros())
    jax.block_until_ready(out)
    if _time_runs:
        times = []
        for _ in range(_time_runs):
            z = zeros()
            jax.block_until_ready(z)
            t0 = time.perf_counter()
            out = r["sharded"](*dev_in, *z)
            jax.block_until_ready(out)
            times.append(time.perf_counter() - t0)
        _cache["exec_times_s"] = times
    yi = r["out_names"].index("y")
    y_all = np.asarray(out[yi]).reshape(8, S, D)
    yf = np.empty((B, S, D), dtype=np.float32)
    for b in range(B):
        yf[b] = y_all[2 * b] + y_all[2 * b + 1]
    return yf
